# revision 9
# baseline (speedup 1.0000x reference)
"""HeadQK kernel for trn2: out = segsum_vocab(causal(q @ k.T / 256)) over 8 cores.

Strategy: cover the causally-active (j-tile, t-chunk) blocks of the T x T
attention matrix c with 8 uniform regions, one per core.  Each region is
NQ=3 t-chunks x NK=8 j-tiles (24 blocks); a block computes
c[j, t] = <k_j, q_t> with k = x @ Wk, q = x @ (Wq/256).  A core computes q
only for its region's 3 chunks and k only for its 8 j-tiles, so the big
projection work is split across cores instead of replicated (regions are
chosen tall-and-narrow because a q-chunk costs 4x a k-tile on the PE).
All matmuls run in bf16 (fp32 PSUM), outputs are written as bf16 raw c
blocks on the fast sync HWDGE queue (one DMA per j-tile), and the host
applies the causal tril mask and the vocab segment-sum (index bookkeeping)
in fp32.  The device program is identical on every core (SPMD); per-core
work differs only through input data.
"""

import sys

import numpy as np

if "/opt/trn_rl_repo" not in sys.path:
    sys.path.insert(0, "/opt/trn_rl_repo")

import ml_dtypes

import concourse.bacc as bacc
import concourse.mybir as mybir
import concourse.tile as tile
from concourse.bass_utils import run_bass_kernel_spmd

T, C, D, V = 4096, 1024, 256, 32000
NCORES = 8
NCH = 8            # t chunks in T
CW = T // NCH      # 512
NQ = 3             # t-chunks per region
NK = 8             # j-tiles per region
HK = NK // 2       # j-tiles per xkt half
CT = C // 128      # 8 contraction tiles
DT = D // 128      # 2 d tiles
F32 = mybir.dt.float32
BF16 = mybir.dt.bfloat16
BF = ml_dtypes.bfloat16

# core p computes blocks (g, ch) for ch in REGIONS[p][0], g in REGIONS[p][1];
# together the regions cover every causally-active block (ch >= g//4).
REGIONS = [
    ([7, 6, 5], [0, 1, 2, 3, 4, 5, 6, 7]),
    ([7, 6, 5], [8, 9, 10, 11, 12, 13, 14, 15]),
    ([7, 6, 5], [16, 17, 18, 19, 20, 21, 22, 23]),
    ([7, 6, 4], [24, 25, 26, 27, 28, 29, 30, 31]),
    ([4, 3, 2], [0, 1, 2, 3, 4, 5, 6, 7]),
    ([4, 3, 2], [8, 9, 10, 11, 12, 13, 14, 15]),
    ([4, 1, 0], [16, 17, 18, 19, 4, 5, 6, 7]),
    ([1, 0, 3], [0, 1, 2, 3, 4, 5, 6, 7]),
]


def _build():
    nc = bacc.Bacc("TRN2", target_bir_lowering=False, debug=False,
                   num_devices=NCORES)
    wk = nc.dram_tensor("wk", [128, CT * D], BF16, kind="ExternalInput")
    xk0 = nc.dram_tensor("xk0", [128, CT * CW], BF16, kind="ExternalInput")
    xk1 = nc.dram_tensor("xk1", [128, CT * CW], BF16, kind="ExternalInput")
    wq = nc.dram_tensor("wq", [128, CT * D], BF16, kind="ExternalInput")
    xh = nc.dram_tensor("xh", [NQ, 128, CT * CW], BF16, kind="ExternalInput")
    out = nc.dram_tensor("out", [NK, 128, NQ * CW], BF16,
                         kind="ExternalOutput")

    with tile.TileContext(nc) as tc:
        with (
            tc.tile_pool(name="const", bufs=1) as cpool,
            tc.tile_pool(name="obuf", bufs=3) as opool,
            tc.tile_pool(name="psq", bufs=2, space="PSUM") as psq,
            tc.tile_pool(name="psk", bufs=2, space="PSUM") as psk,
            tc.tile_pool(name="psc", bufs=4, space="PSUM") as psc,
        ):
            # DMA order = need order: the k half-0 c8=0,1 matmuls only need the
            # first slices of wk + xk0, so split those loads to start the PE
            # as early as possible.
            wk_b = cpool.tile([128, CT * D], BF16)
            nc.sync.dma_start(out=wk_b[:, 0:2 * D], in_=wk[:, 0:2 * D])
            xk_b = []
            xk_h = cpool.tile([128, CT * CW], BF16, tag="xk0")
            nc.sync.dma_start(out=xk_h[:, 0:2 * CW], in_=xk0[:, 0:2 * CW])
            nc.sync.dma_start(out=wk_b[:, 2 * D:], in_=wk[:, 2 * D:])
            nc.sync.dma_start(out=xk_h[:, 2 * CW:], in_=xk0[:, 2 * CW:])
            xk_b.append(xk_h)
            xk_h1 = cpool.tile([128, CT * CW], BF16, tag="xk1")
            nc.sync.dma_start(out=xk_h1[:], in_=xk1[:])
            xk_b.append(xk_h1)
            wq_b = cpool.tile([128, CT * D], BF16)
            nc.sync.dma_start(out=wq_b[:], in_=wq[:])
            xh_b = []
            for i in range(NQ):
                xt = cpool.tile([128, CT * CW], BF16, tag=f"xh{i}")
                nc.sync.dma_start(out=xt[:], in_=xh[i])
                xh_b.append(xt)

            # warm-up: dummy matmuls on a zeroed scratch tile keep the PE busy
            # while the first input DMAs are in flight, so the HAM clock gate
            # reaches full rate before the real matmuls start
            warm = cpool.tile([128, CW], BF16)
            nc.vector.memset(warm[:], 0.0)
            for w in range(24):
                wp = psc.tile([128, CW], F32, tag="cp")
                nc.tensor.matmul(out=wp[:, 0:128], lhsT=warm[:, 0:128],
                                 rhs=warm[:, 0:128], start=True, stop=True)

            # kT pass: ktb[d][dp, tt*128 + jj] = k[tile_tt j=jj, d*128 + dp]
            ktb = []
            for d in range(DT):
                kt = cpool.tile([128, NK * 128], BF16, tag=f"kt{d}")
                ktb.append(kt)
            ncast = [0]

            def psum_to_sbuf(dst, src):
                # alternate scalar/vector so neither engine becomes the
                # bottleneck for the PSUM->SBUF cast traffic
                if ncast[0] % 2 == 0:
                    nc.scalar.copy(out=dst, in_=src)
                else:
                    nc.vector.tensor_copy(out=dst, in_=src)
                ncast[0] += 1

            for h in range(2):
                for d in range(DT):
                    kp = psk.tile([128, CW], F32, tag="kp")
                    for c8 in range(CT):
                        nc.tensor.matmul(
                            out=kp[:],
                            lhsT=wk_b[:, c8 * D + d * 128:
                                      c8 * D + (d + 1) * 128],
                            rhs=xk_b[h][:, c8 * CW:(c8 + 1) * CW],
                            start=(c8 == 0), stop=(c8 == CT - 1),
                        )
                    psum_to_sbuf(ktb[d][:, h * CW:(h + 1) * CW], kp[:])

            qt = []
            for d in range(DT):
                qt_d = cpool.tile([128, NQ * CW], BF16, tag=f"qt{d}")
                qt.append(qt_d)

            for i in range(NQ):
                for d in range(DT):
                    qp = psq.tile([128, CW], F32, tag="qp")
                    for c8 in range(CT):
                        nc.tensor.matmul(
                            out=qp[:],
                            lhsT=wq_b[:, c8 * D + d * 128:
                                      c8 * D + (d + 1) * 128],
                            rhs=xh_b[i][:, c8 * CW:(c8 + 1) * CW],
                            start=(c8 == 0), stop=(c8 == CT - 1),
                        )
                    psum_to_sbuf(qt[d][:, i * CW:(i + 1) * CW], qp[:])

            # c blocks, j-tile major: all NQ chunks of a tile land in one
            # SBUF tile and leave as a single output DMA on the sync queue
            for tt in range(NK):
                ob = opool.tile([128, NQ * CW], BF16, tag="ob")
                for i in range(NQ):
                    cp = psc.tile([128, CW], F32, tag="cp")
                    for d in range(DT):
                        nc.tensor.matmul(
                            out=cp[:],
                            lhsT=ktb[d][:, tt * 128:(tt + 1) * 128],
                            rhs=qt[d][:, i * CW:(i + 1) * CW],
                            start=(d == 0), stop=(d == DT - 1),
                        )
                    psum_to_sbuf(ob[:, i * CW:(i + 1) * CW], cp[:])
                    if tt == NK - 1:
                        # last tile: per-chunk writes shorten the final tail
                        nc.sync.dma_start(
                            out=out[tt][:, i * CW:(i + 1) * CW],
                            in_=ob[:, i * CW:(i + 1) * CW])
                if tt < NK - 1:
                    nc.sync.dma_start(out=out[tt], in_=ob[:])
    nc.compile()
    return nc


def kernel(x, idx, Wq, Wk):
    x = np.asarray(x, dtype=np.float32)
    idx = np.asarray(idx).astype(np.int64)
    Wq = np.asarray(Wq, dtype=np.float32)
    Wk = np.asarray(Wk, dtype=np.float32)

    xb = x.astype(BF)
    # xh_all[ch, cin, c8*CW + tin] = x[ch*CW + tin, c8*128 + cin]
    xh_all = np.ascontiguousarray(
        xb.reshape(NCH, CW, CT, 128).transpose(0, 3, 2, 1)
        .reshape(NCH, 128, CT * CW))
    wq2 = np.ascontiguousarray(
        (Wq / 256.0).astype(BF).reshape(CT, 128, D).transpose(1, 0, 2)
        .reshape(128, CT * D))
    wk2 = np.ascontiguousarray(
        Wk.astype(BF).reshape(CT, 128, D).transpose(1, 0, 2)
        .reshape(128, CT * D))

    in_maps = []
    for p in range(NCORES):
        chunks, tiles = REGIONS[p]
        xks = []
        for h in range(2):
            rows = np.concatenate(
                [np.arange(g * 128, (g + 1) * 128)
                 for g in tiles[h * HK:(h + 1) * HK]])
            # xk[cin, c8*CW + tt*128 + jj] = x[rows[tt*128+jj], c8*128+cin]
            xs = xb[rows]                              # [HK*128, C]
            xks.append(np.ascontiguousarray(
                xs.reshape(HK * 128, CT, 128).transpose(2, 1, 0)
                .reshape(128, CT * HK * 128)))
        xh = np.ascontiguousarray(xh_all[chunks])      # [NQ, 128, CT*CW]
        in_maps.append({"xh": xh, "xk0": xks[0], "xk1": xks[1],
                        "wq": wq2, "wk": wk2})

    nc = _build()
    res = run_bass_kernel_spmd(nc, in_maps, core_ids=list(range(NCORES)))

    # assemble c [T(j), T(t)] in fp32 from the active blocks of each region,
    # apply the causal mask, segment-sum over j -> vocab on the host
    cmat = np.zeros((T, T), np.float32)
    for p in range(NCORES):
        chunks, tiles = REGIONS[p]
        blk = np.asarray(res.results[p]["out"]).astype(np.float32)
        for tt, g in enumerate(tiles):
            for qq, ch in enumerate(chunks):
                if ch >= g // 4:     # causally active block
                    cmat[g * 128:(g + 1) * 128, ch * CW:(ch + 1) * CW] = \
                        blk[tt, :, qq * CW:(qq + 1) * CW]
    jj = np.arange(T)
    cmat *= jj[None, :] >= jj[:, None]      # keep t >= j
    order = np.argsort(idx, kind="stable")
    sidx = idx[order]
    starts = np.flatnonzero(np.r_[True, sidx[1:] != sidx[:-1]])
    red = np.add.reduceat(cmat[order], starts, axis=0)  # [nu, T]
    outf = np.zeros((T, V), np.float32)
    outf[:, sidx[starts]] = red.T
    return outf
